# revision 20
# baseline (speedup 1.0000x reference)
"""PVT-style spatial-reduction attention on 8 Trainium2 NeuronCores.

Sharding: data-parallel over batch (B=8 -> one batch element per core).
Each core runs the full attention for its batch element; weights are
replicated. No collectives needed.

v2 schedule (ACT-exp is the bottleneck engine; everything hides behind it):
  - x^T arrives as 12 pipelined DMA-transpose pieces on both HWDGE rings.
  - conv2x2s2 contracts strided views of x^T directly as matmul lhsT
    (no patch materialization).
  - attention starts after ~1/4 of the prep (conv blocks 0-3, kT chunk 0,
    qT rows 0:128); remaining conv/lnT/v/kT/qproj/proj work is drip-fed
    into PE slack between score/av matmuls via a prep queue.
  - scores psum ping-pongs in 2x[128,1024] banks; av accumulates in
    2x[128,512]; per-head softmax normalization is taken off the critical
    path by releasing the av psum with a fast reciprocal + value copy,
    then broadcasting/multiplying lazily.
"""

import os
import sys
from collections import deque

import numpy as np

for _p in ("/opt/trn_rl_repo", "/root/.axon_site/_ro/trn_rl_repo"):
    if os.path.isdir(_p) and _p not in sys.path:
        sys.path.append(_p)

import concourse.bacc as bacc
import concourse.bass as bass
import concourse.mybir as mybir
import concourse.tile as tile
from concourse.bass_utils import run_bass_kernel_spmd
from concourse.masks import make_identity

F16 = mybir.dt.float16
F32 = mybir.dt.float32

N = 4096          # q tokens (H*W = 64*64)
C = 320           # model dim
NH = 5            # heads
HD = 64           # head dim
NP = 1024         # kv tokens ((H/2)*(W/2))
QB = 512
LN_EPS = 1e-3
SCALE = HD ** -0.5
EXP_BIAS = -3.0   # constant shift inside exp; cancels in softmax

# contraction chunks over C=320: three 128-row tiles; the last one holds
# c 192:320 and uses rows 64:128 (its top 64 rows overlap chunk 1).
CCHUNKS = [(0, 0, 128), (128, 0, 128), (192, 64, 128)]  # (c_start, row0, rows)
# output chunks over C=320
OCHUNKS = [(0, 128), (128, 128), (256, 64)]


def build_bass(dbg=False):
    nc = bacc.Bacc("TRN2", target_bir_lowering=False, debug=False, num_devices=8)

    x_d = nc.declare_dram_parameter("x", [N, C], F16, isOutput=False)
    xd_d = nc.declare_dram_parameter("xd", [N, C], F16, isOutput=False)
    wq_d = nc.declare_dram_parameter("wq", [C, C], F16, isOutput=False)
    wk_d = nc.declare_dram_parameter("wk", [C, C], F16, isOutput=False)
    wv_d = nc.declare_dram_parameter("wv", [C, C], F16, isOutput=False)
    srw_d = nc.declare_dram_parameter("srw", [4 * C, C], F16, isOutput=False)
    wp_d = nc.declare_dram_parameter("wp", [C, C], F16, isOutput=False)
    srb_d = nc.declare_dram_parameter("srb", [C], F32, isOutput=False)
    bk_d = nc.declare_dram_parameter("bk", [C], F32, isOutput=False)
    bv_d = nc.declare_dram_parameter("bv", [C], F32, isOutput=False)
    bp_d = nc.declare_dram_parameter("bp", [C], F32, isOutput=False)
    out_d = nc.declare_dram_parameter("out", [N, C], F32, isOutput=True)
    dbg_d = {}
    if dbg:
        for nm, shp in [("dbg_xt0", [128, N]), ("dbg_xtd0", [128, N]),
                        ("dbg_ln0", [128, NP]), ("dbg_kt0", [128, NP]),
                        ("dbg_qt0", [128, N]), ("dbg_v", [128, 8 * NH * 128]),
                        ("dbg_se", [128, 1024]), ("dbg_at", [128, 1024])]:
            dbg_d[nm] = nc.declare_dram_parameter(nm, shp, F16, isOutput=True)

    with tile.TileContext(nc) as tc:
        with (
            tc.tile_pool(name="consts", bufs=1) as consts,
            tc.tile_pool(name="wpool", bufs=1) as wpool,
            tc.tile_pool(name="big", bufs=1) as bigp,
            tc.tile_pool(name="sexp", bufs=18) as sexp_p,
            tc.tile_pool(name="attn", bufs=2) as attn_p,
            tc.tile_pool(name="small", bufs=4) as small_p,
            tc.tile_pool(name="vcop", bufs=4) as vcop_p,
            tc.tile_pool(name="outp", bufs=4) as out_p,
            tc.tile_pool(name="ps_s", bufs=2, space="PSUM") as ps_s,
            tc.tile_pool(name="ps_a", bufs=2, space="PSUM") as ps_a,
            tc.tile_pool(name="ps_m", bufs=2, space="PSUM") as ps_m,
        ):
            # ---------------- DMA: x^T pieces + weights ----------------
            # x^T tiles (c on partitions), filled by 12 transpose pieces
            # (3 c-chunks x 4 token-quarters) so conv/qproj can start after
            # the first quarter instead of after the whole 14us transpose.
            # xTd is the shift-deinterleaved copy (host-permuted xd input,
            # rows = [it, dh, dw, h'%4, w']): the conv's stationary operand
            # becomes a plain contiguous 2D slice (PE weights allow only one
            # free dim). xTd pieces go on the sync ring, xT on the ACT ring.
            # per-piece tiles (full-tile transpose writes; one DMA ring —
            # concurrent xbar transposes on two rings corrupted data).
            # xTp[ci][sp] holds tokens [sp*1024, (sp+1)*1024) of x^T chunk ci.
            xTp = [[bigp.tile([128, 1024], F16, name=f"xT{i}_{sp}")
                    for sp in range(4)] for i in range(3)]
            xTdp = [[bigp.tile([128, 1024], F16, name=f"xTd{i}_{sp}")
                     for sp in range(4)] for i in range(3)]

            def xtd_piece(sp):
                for ci, (c0, _r0, _rows) in enumerate(CCHUNKS):
                    nc.sync.dma_start_transpose(
                        out=xTdp[ci][sp],
                        in_=xd_d[sp * 1024:(sp + 1) * 1024, c0:c0 + 128])

            def xt_piece(sp):
                for ci, (c0, _r0, _rows) in enumerate(CCHUNKS):
                    nc.sync.dma_start_transpose(
                        out=xTp[ci][sp],
                        in_=x_d[sp * 1024:(sp + 1) * 1024, c0:c0 + 128])

            xtd_piece(0)
            xtd_piece(1)
            xt_piece(0)
            xtd_piece(2)
            xtd_piece(3)

            # weights share the second HWDGE ring (ACT engine issues) with
            # the xT pieces so both stream concurrently with xTd.
            # sr_w in 12 per-(shift, cchunk) tiles; chunk 2 parks its 64
            # rows at partitions 64:128 so matmul bases match the xTd slice.
            srw_sb = []
            for s in range(4):
                per_s = []
                for ci, (c0, r0, rows) in enumerate(CCHUNKS):
                    t = wpool.tile([128, C], F16, name=f"srw{s}_{ci}")
                    nc.scalar.dma_start(
                        out=t[r0:128, :],
                        in_=srw_d[s * C + c0 + r0:s * C + c0 + 128, :])
                    per_s.append(t)
                srw_sb.append(per_s)

            def load_w_chunks(dram, name):
                ts = []
                for i, (c0, _r0, rows) in enumerate(CCHUNKS):
                    t = wpool.tile([rows, C], F16, name=f"{name}{i}")
                    nc.scalar.dma_start(out=t, in_=dram[c0:c0 + rows, :])
                    ts.append(t)
                return ts

            xt_piece(0)
            wq_sb = load_w_chunks(wq_d, "wq")
            xt_piece(1)
            wk_sb = load_w_chunks(wk_d, "wk")
            wv_sb = load_w_chunks(wv_d, "wv")
            xt_piece(2)
            xt_piece(3)
            wp_o = []
            bk_col = []
            for i, (o0, osz) in enumerate(OCHUNKS):
                t = wpool.tile([osz, C], F16, name=f"wp{i}")
                nc.scalar.dma_start(out=t, in_=wp_d[o0:o0 + osz, :])
                wp_o.append(t)
                b = wpool.tile([osz, 1], F32, name=f"bk{i}")
                nc.scalar.dma_start(out=b, in_=bk_d[o0:o0 + osz].unsqueeze(1))
                bk_col.append(b)

            def bcast(dram_vec, name):
                t = consts.tile([128, C], F32, name=name)
                src = bass.AP(tensor=dram_vec.ap().tensor, offset=0,
                              ap=[[0, 128], [1, C]])
                nc.scalar.dma_start(out=t, in_=src)
                return t

            srb_bc = bcast(srb_d, "srb_bc")
            bv_bc = bcast(bv_d, "bv_bc")
            bp_bc = bcast(bp_d, "bp_bc")

            ident = consts.tile([128, 128], F16, name="ident")
            make_identity(nc, ident)
            eps_t = consts.tile([128, 1], F32, name="eps_t")
            nc.vector.memset(eps_t, LN_EPS)
            ebias_t = consts.tile([128, 1], F32, name="ebias_t")
            nc.vector.memset(ebias_t, EXP_BIAS)

            # warm the ACT exp table set during the ramp so the ~2.7us
            # table load doesn't land inside the attention phase.
            warm = small_p.tile([128, 1], F16, name="warm", tag="st")
            nc.scalar.activation(warm, eps_t,
                                 mybir.ActivationFunctionType.Exp)

            # v augmented: [128, kv_chunk(8), head(5), 128] with ones col 0
            # (softmax denominators land on psum partition 0), zeros 1:64,
            # v at 64:128.
            v_aug = bigp.tile([128, 8, NH, 128], F16, name="v_aug")
            nc.vector.memset(v_aug[:, :, :, 0:64], 0.0)
            nc.vector.memset(v_aug[:, :, :, 0:1], 1.0)

            lnT = [bigp.tile([128, NP], F16, name=f"lnT{i}") for i in range(3)]
            kT = [bigp.tile([osz, NP], F16, name=f"kT{i}")
                  for i, (_o0, osz) in enumerate(OCHUNKS)]
            qT = [bigp.tile([osz, N], F16, name=f"qT{i}")
                  for i, (_o0, osz) in enumerate(OCHUNKS)]

            ln_tiles = [None] * 8

            # ---------------- prep building blocks ----------------
            def conv_group(it, s):
                """One shift (dh,dw) of conv block it: 3 accumulating mms.
                xTd columns are [it(8), shift(4), tok'(128)] so the
                stationary operand is a contiguous 2D slice."""
                if s == 0:
                    conv_group.pc = ps_m.tile([128, C], F32, name="pc", tag="m")
                pc = conv_group.pc
                t0 = it * 512 + s * 128
                sp, tc0 = t0 // 1024, t0 % 1024
                for ci, (_c0, r0, rows) in enumerate(CCHUNKS):
                    nc.tensor.matmul(pc, xTdp[ci][sp][r0:128, tc0:tc0 + 128],
                                     srw_sb[s][ci][r0:128, :],
                                     start=(s == 0 and ci == 0),
                                     stop=(s == 3 and ci == 2))
                if s == 3:
                    ln_stats(it, pc)

            def ln_stats(it, pc):
                nc.vector.tensor_add(pc, pc, srb_bc)
                stats = small_p.tile([128, 6], F32, name="stats", tag="st")
                nc.vector.bn_stats(stats, pc)
                mv = small_p.tile([128, 2], F32, name="mv", tag="st")
                nc.vector.bn_aggr(mv, stats)
                std = small_p.tile([128, 1], F32, name="std", tag="st")
                nc.scalar.activation(std, mv[:, 1:2],
                                     mybir.ActivationFunctionType.Sqrt,
                                     bias=eps_t)
                rstd = small_p.tile([128, 1], F32, name="rstd", tag="st")
                nc.vector.reciprocal(rstd, std)
                ln_h = small_p.tile([128, C], F16, name="ln_h", tag="lnf")
                nc.vector.tensor_scalar(ln_h, pc, mv[:, 0:1], rstd,
                                        op0=mybir.AluOpType.subtract,
                                        op1=mybir.AluOpType.mult)
                ln_tiles[it] = ln_h

            def emit_lnT(it):
                ln_h = ln_tiles[it]
                for ci, (c0, _r0, _rows) in enumerate(CCHUNKS):
                    pt = ps_m.tile([128, 128], F16, name="pt", tag="m")
                    nc.tensor.transpose(pt, ln_h[:, c0:c0 + 128], ident)
                    nc.vector.tensor_copy(lnT[ci][:, it * 128:(it + 1) * 128],
                                          pt)

            def emit_v(it):
                pv = ps_m.tile([128, C], F32, name="pv", tag="m")
                for ci, (_c0, r0, rows) in enumerate(CCHUNKS):
                    nc.tensor.matmul(pv, lnT[ci][r0:128, it * 128:(it + 1) * 128],
                                     wv_sb[ci][r0:128, :],
                                     start=(ci == 0), stop=(ci == 2))
                nc.vector.tensor_add(
                    v_aug[:, it, :, 64:],
                    pv.rearrange("p (h d) -> p h d", h=NH),
                    bv_bc.rearrange("p (h d) -> p h d", h=NH))

            def emit_kT(i, b, tag, w=QB):
                """kT[i] columns [b*w, (b+1)*w)."""
                o0, osz = OCHUNKS[i]
                pk = ps_s.tile([osz, w], F32, name="pk", tag=tag) if tag == "s" \
                    else ps_m.tile([osz, w], F32, name="pk", tag=tag)
                for ci, (_c0, r0, rows) in enumerate(CCHUNKS):
                    nc.tensor.matmul(
                        pk, wk_sb[ci][r0:128, o0:o0 + osz],
                        lnT[ci][r0:128, b * w:(b + 1) * w],
                        start=(ci == 0), stop=(ci == 2))
                nc.vector.tensor_scalar_add(
                    kT[i][:, b * w:(b + 1) * w], pk, bk_col[i])

            def emit_qproj(i, nb, tag):
                o0, osz = OCHUNKS[i]
                pq = ps_s.tile([osz, QB], F32, name="pq", tag=tag) if tag == "s" \
                    else ps_m.tile([osz, QB], F32, name="pq", tag=tag)
                sp, tc0 = (nb * QB) // 1024, (nb * QB) % 1024
                for ci, (_c0, r0, rows) in enumerate(CCHUNKS):
                    nc.tensor.matmul(
                        pq, wq_sb[ci][r0:128, o0:o0 + osz],
                        xTp[ci][sp][r0:128, tc0:tc0 + QB],
                        start=(ci == 0), stop=(ci == 2))
                nc.vector.tensor_copy(qT[i][:, nb * QB:(nb + 1) * QB], pq)

            # ---------------- attention building blocks ----------------
            attnT = {}

            def emit_scores(qb, h, k):
                ht, hr = h // 2, (h % 2) * 64
                ps = ps_s.tile([128, 2 * QB], F32, name="ps", tag="s")
                for qh in range(2):
                    nc.tensor.matmul(
                        ps[:, qh * QB:(qh + 1) * QB],
                        kT[ht][hr:hr + HD, k * 128:(k + 1) * 128],
                        qT[ht][hr:hr + HD,
                               qb * 1024 + qh * QB:qb * 1024 + (qh + 1) * QB],
                        start=True, stop=True)
                se = sexp_p.tile([128, 2 * QB], F16, name="se", tag="sexp")
                nc.scalar.activation(se, ps, mybir.ActivationFunctionType.Exp,
                                     bias=ebias_t, scale=SCALE)
                if dbg and qb == 0 and h == 0 and k == 0:
                    nc.sync.dma_start(out=dbg_d["dbg_se"][:, :], in_=se)
                return se

            def emit_av(pavs, h, k, se):
                for qh in range(2):
                    nc.tensor.matmul(
                        pavs[qh], v_aug[:, k, h, :],
                        se[:, qh * QB:(qh + 1) * QB],
                        start=(k == 0), stop=(k == 7))

            def emit_norm(qb, h, pavs):
                """Release pav fast (reciprocal + value copy), then lazily
                broadcast+multiply into attnT."""
                dst = attnT[qb][h // 2]
                dr = (h % 2) * 64
                for qh in range(2):
                    rec = small_p.tile([1, QB], F32, name="rec", tag="rc")
                    nc.vector.reciprocal_approx_fast(rec, pavs[qh][0:1, :])
                    vcp = vcop_p.tile([64, QB], F16, name="vcp", tag="vc")
                    nc.vector.tensor_copy(vcp, pavs[qh][64:128, :])
                    rb = small_p.tile([HD, QB], F32, name="rb", tag="rb")
                    nc.gpsimd.partition_broadcast(rb, rec)
                    nc.vector.tensor_mul(
                        dst[dr:dr + HD, qh * QB:(qh + 1) * QB], vcp, rb)

            def emit_proj_qs(qb, qs):
                po = ps_m.tile([128, C], F32, name="po", tag="m")
                for ci, (o0, osz) in enumerate(OCHUNKS):
                    nc.tensor.matmul(
                        po, attnT[qb][ci][:, qs * 128:(qs + 1) * 128],
                        wp_o[ci], start=(ci == 0), stop=(ci == 2))
                o_sb = out_p.tile([128, C], F32, name="o_sb", tag="o")
                nc.vector.tensor_add(o_sb, po, bp_bc)
                nc.sync.dma_start(
                    out=out_d[(qb * 8 + qs) * 128:(qb * 8 + qs + 1) * 128, :],
                    in_=o_sb)

            # ---------------- prep queue ----------------
            prep = deque()

            def pump(n):
                for _ in range(n):
                    if prep:
                        prep.popleft()()

            def prep_block(it):
                # conv block it as 5 queue items: 4 shift groups + (lnT+v)
                for s in range(4):
                    prep.append(lambda it=it, s=s: conv_group(it, s))
                prep.append(lambda it=it: (emit_lnT(it), emit_v(it)))

            # ---------------- ramp ----------------
            for it in range(4):
                for s in range(4):
                    conv_group(it, s)
            for it in range(4):
                emit_lnT(it)
                emit_v(it)
            emit_kT(0, 0, "s")
            emit_qproj(0, 0, "s")
            emit_qproj(0, 1, "s")

            # remaining prep, in dependency-safe pump order
            for it in range(4, 8):
                prep_block(it)
            prep.append(lambda: (emit_kT(1, 0, "m"), emit_kT(1, 1, "m")))
            prep.append(lambda: (emit_qproj(1, 0, "m"), emit_qproj(1, 1, "m")))
            prep.append(lambda: (emit_kT(2, 0, "m"), emit_kT(2, 1, "m")))
            prep.append(lambda: (emit_qproj(2, 0, "m"), emit_qproj(2, 1, "m")))
            for nb in range(2, 8):
                for i in range(3):
                    prep.append(lambda i=i, nb=nb: emit_qproj(i, nb, "m"))

            # ---------------- attention ----------------
            for qb in range(4):
                attnT[qb] = [
                    attn_p.tile([osz, 1024], F16, name=f"aT{qb}_{i}",
                                tag=f"attn{i}")
                    for i, (_o0, osz) in enumerate(OCHUNKS)]
                for h in range(NH):
                    pavs = [ps_a.tile([128, QB], F32, name="pav", tag="a")
                            for _ in range(2)]
                    ses = {}
                    if qb == 0 and h == 0:
                        # special pacing: kv chunks 4..7 need conv(4..7),
                        # lnT/v(k) and the kT[0] 128-col piece first.
                        for k in range(4):
                            ses[k] = emit_scores(qb, h, k)
                            if k >= 1:
                                emit_av(pavs, h, k - 1, ses[k - 1])
                        for k in range(4, 8):
                            pump(5)        # conv block k + lnT/v(k)
                            emit_kT(0, k, "m", w=128)
                            ses[k] = emit_scores(qb, h, k)
                            emit_av(pavs, h, k - 1, ses[k - 1])
                        emit_av(pavs, h, 7, ses[7])
                    else:
                        for k in range(8):
                            ses[k] = emit_scores(qb, h, k)
                            if k >= 1:
                                emit_av(pavs, h, k - 1, ses[k - 1])
                            if k % 2 == 1:
                                pump(1)
                        emit_av(pavs, h, 7, ses[7])
                    emit_norm(qb, h, pavs)
                    pump(1)
                if dbg and qb == 0:
                    nc.sync.dma_start(out=dbg_d["dbg_at"][:, :], in_=attnT[0][0])
                    for sp in range(4):
                        nc.sync.dma_start(
                            out=dbg_d["dbg_xt0"][:, sp * 1024:(sp + 1) * 1024],
                            in_=xTp[0][sp])
                        nc.sync.dma_start(
                            out=dbg_d["dbg_xtd0"][:, sp * 1024:(sp + 1) * 1024],
                            in_=xTdp[0][sp])
                    nc.sync.dma_start(out=dbg_d["dbg_ln0"][:, :], in_=lnT[0])
                    nc.sync.dma_start(out=dbg_d["dbg_kt0"][0:128, :], in_=kT[0])
                    nc.sync.dma_start(
                        out=dbg_d["dbg_v"][:, :],
                        in_=v_aug.rearrange("p a b c -> p (a b c)"))
                if qb < 3:
                    for qs in range(8):
                        prep.append(lambda qb=qb, qs=qs: emit_proj_qs(qb, qs))
            pump(len(prep))
            for qs in range(8):
                emit_proj_qs(3, qs)
            if dbg:
                nc.sync.dma_start(out=dbg_d["dbg_qt0"][:, :], in_=qT[0])

    nc.compile()
    return nc


_CACHE = {}


def _get_nc():
    if "nc" not in _CACHE:
        _CACHE["nc"] = build_bass()
    return _CACHE["nc"]


def make_in_maps(x, Wq, Wkv, sr_w, sr_b, ln_g, ln_b, Wp, bp):
    B = x.shape[0]
    f16 = np.float16
    f32 = np.float32
    ln_g = np.asarray(ln_g, f32)
    ln_b = np.asarray(ln_b, f32)
    wk_f = np.asarray(Wkv[:, :C], f32)
    wv_f = np.asarray(Wkv[:, C:], f32)
    wq = np.ascontiguousarray(Wq, dtype=f16)
    # fold LN gamma/beta into the K/V projections:
    #   (ln*g + b) @ W = ln @ (g[:,None]*W) + b @ W
    wk = np.ascontiguousarray(ln_g[:, None] * wk_f, dtype=f16)
    wv = np.ascontiguousarray(ln_g[:, None] * wv_f, dtype=f16)
    bk = np.ascontiguousarray(ln_b @ wk_f, dtype=f32)
    bv = np.ascontiguousarray(ln_b @ wv_f, dtype=f32)
    srw = np.ascontiguousarray(np.asarray(sr_w, dtype=f16).reshape(4 * C, C))
    wp = np.ascontiguousarray(Wp, dtype=f16)
    srb = np.ascontiguousarray(sr_b, dtype=f32)
    bpv = np.ascontiguousarray(bp, dtype=f32)
    # shift-deinterleaved copy of x for the conv's stationary operand:
    # row order [it(h'//4), dh, dw, h'%4, w'] <- x row (2h'+dh)*64 + 2w'+dw
    xf = np.asarray(x, dtype=f16)
    xd = (xf.reshape(B, 8, 4, 2, 32, 2, C)        # [B, it, h'lo, dh, w', dw, C]
          .transpose(0, 1, 3, 5, 2, 4, 6)          # [B, it, dh, dw, h'lo, w', C]
          .reshape(B, N, C))
    return [
        {"x": np.ascontiguousarray(xf[i]), "xd": np.ascontiguousarray(xd[i]),
         "wq": wq, "wk": wk,
         "wv": wv, "srw": srw, "wp": wp, "srb": srb, "bk": bk,
         "bv": bv, "bp": bpv}
        for i in range(B)
    ]


def kernel(x, Wq, Wkv, sr_w, sr_b, ln_g, ln_b, Wp, bp, H=64, W=64):
    x = np.asarray(x, dtype=np.float32)
    B = x.shape[0]
    assert x.shape == (B, N, C), x.shape
    nc = _get_nc()
    in_maps = make_in_maps(x, Wq, Wkv, sr_w, sr_b, ln_g, ln_b, Wp, bp)
    res = run_bass_kernel_spmd(nc, in_maps, core_ids=list(range(8)))
    out = np.stack([res.results[i]["out"] for i in range(B)], axis=0)
    return out.astype(np.float32)


# revision 23
# speedup vs baseline: 1.2075x; 1.2075x over previous
"""PVT-style spatial-reduction attention on 8 Trainium2 NeuronCores.

Sharding: data-parallel over batch (B=8 -> one batch element per core).
Each core runs the full attention for its batch element; weights are
replicated. No collectives needed.

v2 schedule (ACT-exp is the bottleneck engine; everything hides behind it):
  - x^T arrives as 12 pipelined DMA-transpose pieces on both HWDGE rings.
  - conv2x2s2 contracts strided views of x^T directly as matmul lhsT
    (no patch materialization).
  - attention starts after ~1/4 of the prep (conv blocks 0-3, kT chunk 0,
    qT rows 0:128); remaining conv/lnT/v/kT/qproj/proj work is drip-fed
    into PE slack between score/av matmuls via a prep queue.
  - scores psum ping-pongs in 2x[128,1024] banks; av accumulates in
    2x[128,512]; per-head softmax normalization is taken off the critical
    path by releasing the av psum with a fast reciprocal + value copy,
    then broadcasting/multiplying lazily.
"""

import os
import sys
from collections import deque

import numpy as np

for _p in ("/opt/trn_rl_repo", "/root/.axon_site/_ro/trn_rl_repo"):
    if os.path.isdir(_p) and _p not in sys.path:
        sys.path.append(_p)

import concourse.bacc as bacc
import concourse.bass as bass
import concourse.mybir as mybir
import concourse.tile as tile
from concourse.bass_utils import run_bass_kernel_spmd
from concourse.masks import make_identity

F16 = mybir.dt.float16
F32 = mybir.dt.float32

N = 4096          # q tokens (H*W = 64*64)
C = 320           # model dim
NH = 5            # heads
HD = 64           # head dim
NP = 1024         # kv tokens ((H/2)*(W/2))
QB = 512
LN_EPS = 1e-3
SCALE = HD ** -0.5
EXP_BIAS = -3.0   # constant shift inside exp; cancels in softmax

# contraction chunks over C=320: three 128-row tiles; the last one holds
# c 192:320 and uses rows 64:128 (its top 64 rows overlap chunk 1).
CCHUNKS = [(0, 0, 128), (128, 0, 128), (192, 64, 128)]  # (c_start, row0, rows)
# output chunks over C=320
OCHUNKS = [(0, 128), (128, 128), (256, 64)]


def build_bass(dbg=False):
    nc = bacc.Bacc("TRN2", target_bir_lowering=False, debug=False, num_devices=8)

    xt_d = nc.declare_dram_parameter("xt", [C, N], F16, isOutput=False)
    xdt_d = nc.declare_dram_parameter("xdt", [C, N], F16, isOutput=False)
    wq_d = nc.declare_dram_parameter("wq", [C, C], F16, isOutput=False)
    wk_d = nc.declare_dram_parameter("wk", [C, C], F16, isOutput=False)
    wv_d = nc.declare_dram_parameter("wv", [C, C], F16, isOutput=False)
    srw_d = nc.declare_dram_parameter("srw", [4 * C, C], F16, isOutput=False)
    wp_d = nc.declare_dram_parameter("wp", [C, C], F16, isOutput=False)
    srb_d = nc.declare_dram_parameter("srb", [C], F32, isOutput=False)
    bk_d = nc.declare_dram_parameter("bk", [C], F32, isOutput=False)
    bv_d = nc.declare_dram_parameter("bv", [C], F32, isOutput=False)
    bp_d = nc.declare_dram_parameter("bp", [C], F32, isOutput=False)
    out_d = nc.declare_dram_parameter("out", [N, C], F32, isOutput=True)
    dbg_d = {}
    if dbg:
        for nm, shp in [("dbg_xt0", [128, N]), ("dbg_xtd0", [128, N]),
                        ("dbg_ln0", [128, NP]), ("dbg_kt0", [128, NP]),
                        ("dbg_qt0", [128, N]), ("dbg_v", [128, 8 * NH * 128]),
                        ("dbg_se", [128, 1024]), ("dbg_at", [128, 1024])]:
            dbg_d[nm] = nc.declare_dram_parameter(nm, shp, F16, isOutput=True)

    with tile.TileContext(nc) as tc:
        with (
            tc.tile_pool(name="consts", bufs=1) as consts,
            tc.tile_pool(name="wpool", bufs=1) as wpool,
            tc.tile_pool(name="big", bufs=1) as bigp,
            tc.tile_pool(name="sexp", bufs=18) as sexp_p,
            tc.tile_pool(name="attn", bufs=2) as attn_p,
            tc.tile_pool(name="small", bufs=4) as small_p,
            tc.tile_pool(name="vcop", bufs=4) as vcop_p,
            tc.tile_pool(name="outp", bufs=4) as out_p,
            tc.tile_pool(name="ps_s", bufs=2, space="PSUM") as ps_s,
            tc.tile_pool(name="ps_a", bufs=2, space="PSUM") as ps_a,
            tc.tile_pool(name="ps_m", bufs=2, space="PSUM") as ps_m,
        ):
            # ---------------- DMA: x^T pieces + weights ----------------
            # x^T tiles (c on partitions), filled by 12 transpose pieces
            # (3 c-chunks x 4 token-quarters) so conv/qproj can start after
            # the first quarter instead of after the whole 14us transpose.
            # xTd is the shift-deinterleaved copy (host-permuted xd input,
            # rows = [it, dh, dw, h'%4, w']): the conv's stationary operand
            # becomes a plain contiguous 2D slice (PE weights allow only one
            # free dim). xTd pieces go on the sync ring, xT on the ACT ring.
            # x^T and the shift-deinterleaved xd^T are pre-transposed on the
            # host, so all input DMAs are plain row-contiguous transfers
            # (on-device xbar transposes ran at ~20 GB/s in 640B packets and
            # starved the ramp). Per-piece tiles keep subtile deps exact.
            # xTp[ci][sp] holds tokens [sp*1024, (sp+1)*1024) of x^T chunk ci.
            xTp = [[bigp.tile([128, 1024], F16, name=f"xT{i}_{sp}")
                    for sp in range(4)] for i in range(3)]
            xTdp = [[bigp.tile([128, 1024], F16, name=f"xTd{i}_{sp}")
                     for sp in range(4)] for i in range(3)]

            def xtd_piece(sp):
                for ci, (c0, _r0, _rows) in enumerate(CCHUNKS):
                    nc.sync.dma_start(
                        out=xTdp[ci][sp],
                        in_=xdt_d[c0:c0 + 128, sp * 1024:(sp + 1) * 1024])

            def xt_piece(sp):
                for ci, (c0, _r0, _rows) in enumerate(CCHUNKS):
                    nc.scalar.dma_start(
                        out=xTp[ci][sp],
                        in_=xt_d[c0:c0 + 128, sp * 1024:(sp + 1) * 1024])

            xtd_piece(0)
            xtd_piece(1)
            xtd_piece(2)
            xtd_piece(3)

            # weights share the second HWDGE ring (ACT engine issues) with
            # the xT pieces so both stream concurrently with xTd.
            # sr_w in 12 per-(shift, cchunk) tiles; chunk 2 parks its 64
            # rows at partitions 64:128 so matmul bases match the xTd slice.
            srw_sb = []
            for s in range(4):
                per_s = []
                for ci, (c0, r0, rows) in enumerate(CCHUNKS):
                    t = wpool.tile([128, C], F16, name=f"srw{s}_{ci}")
                    nc.scalar.dma_start(
                        out=t[r0:128, :],
                        in_=srw_d[s * C + c0 + r0:s * C + c0 + 128, :])
                    per_s.append(t)
                srw_sb.append(per_s)

            def load_w_chunks(dram, name):
                ts = []
                for i, (c0, _r0, rows) in enumerate(CCHUNKS):
                    t = wpool.tile([rows, C], F16, name=f"{name}{i}")
                    nc.scalar.dma_start(out=t, in_=dram[c0:c0 + rows, :])
                    ts.append(t)
                return ts

            xt_piece(0)
            wq_sb = load_w_chunks(wq_d, "wq")
            xt_piece(1)
            wk_sb = load_w_chunks(wk_d, "wk")
            wv_sb = load_w_chunks(wv_d, "wv")
            xt_piece(2)
            xt_piece(3)
            wp_o = []
            bk_col = []
            for i, (o0, osz) in enumerate(OCHUNKS):
                t = wpool.tile([osz, C], F16, name=f"wp{i}")
                nc.scalar.dma_start(out=t, in_=wp_d[o0:o0 + osz, :])
                wp_o.append(t)
                b = wpool.tile([osz, 1], F32, name=f"bk{i}")
                nc.scalar.dma_start(out=b, in_=bk_d[o0:o0 + osz].unsqueeze(1))
                bk_col.append(b)

            def bcast(dram_vec, name):
                t = consts.tile([128, C], F32, name=name)
                src = bass.AP(tensor=dram_vec.ap().tensor, offset=0,
                              ap=[[0, 128], [1, C]])
                nc.scalar.dma_start(out=t, in_=src)
                return t

            srb_bc = bcast(srb_d, "srb_bc")
            bv_bc = bcast(bv_d, "bv_bc")
            bp_bc = bcast(bp_d, "bp_bc")

            ident = consts.tile([128, 128], F16, name="ident")
            make_identity(nc, ident)
            eps_t = consts.tile([128, 1], F32, name="eps_t")
            nc.vector.memset(eps_t, LN_EPS)
            ebias_t = consts.tile([128, 1], F32, name="ebias_t")
            nc.vector.memset(ebias_t, EXP_BIAS)

            # warm the ACT exp table set during the ramp so the ~2.7us
            # table load doesn't land inside the attention phase.
            warm = small_p.tile([128, 1], F16, name="warm", tag="st")
            nc.scalar.activation(warm, eps_t,
                                 mybir.ActivationFunctionType.Exp)

            # v augmented: [128, kv_chunk(8), head(5), 128] with ones col 0
            # (softmax denominators land on psum partition 0), zeros 1:64,
            # v at 64:128.
            v_aug = bigp.tile([128, 8, NH, 128], F16, name="v_aug")
            nc.vector.memset(v_aug[:, :, :, 0:64], 0.0)
            nc.vector.memset(v_aug[:, :, :, 0:1], 1.0)

            lnT = [bigp.tile([128, NP], F16, name=f"lnT{i}") for i in range(3)]
            kT = [bigp.tile([osz, NP], F16, name=f"kT{i}")
                  for i, (_o0, osz) in enumerate(OCHUNKS)]
            qT = [bigp.tile([osz, N], F16, name=f"qT{i}")
                  for i, (_o0, osz) in enumerate(OCHUNKS)]

            ln_tiles = [None] * 8

            # ---------------- prep building blocks ----------------
            def conv_group(it, s):
                """One shift (dh,dw) of conv block it: 3 accumulating mms.
                xTd columns are [it(8), shift(4), tok'(128)] so the
                stationary operand is a contiguous 2D slice."""
                if s == 0:
                    conv_group.pc = ps_m.tile([128, C], F32, name="pc", tag="m")
                pc = conv_group.pc
                t0 = it * 512 + s * 128
                sp, tc0 = t0 // 1024, t0 % 1024
                for ci, (_c0, r0, rows) in enumerate(CCHUNKS):
                    nc.tensor.matmul(pc, xTdp[ci][sp][r0:128, tc0:tc0 + 128],
                                     srw_sb[s][ci][r0:128, :],
                                     start=(s == 0 and ci == 0),
                                     stop=(s == 3 and ci == 2))
                if s == 3:
                    ln_stats(it, pc)

            def ln_stats(it, pc):
                nc.vector.tensor_add(pc, pc, srb_bc)
                stats = small_p.tile([128, 6], F32, name="stats", tag="st")
                nc.vector.bn_stats(stats, pc)
                mv = small_p.tile([128, 2], F32, name="mv", tag="st")
                nc.vector.bn_aggr(mv, stats)
                std = small_p.tile([128, 1], F32, name="std", tag="st")
                nc.scalar.activation(std, mv[:, 1:2],
                                     mybir.ActivationFunctionType.Sqrt,
                                     bias=eps_t)
                rstd = small_p.tile([128, 1], F32, name="rstd", tag="st")
                nc.vector.reciprocal(rstd, std)
                ln_h = small_p.tile([128, C], F16, name="ln_h", tag="lnf")
                nc.vector.tensor_scalar(ln_h, pc, mv[:, 0:1], rstd,
                                        op0=mybir.AluOpType.subtract,
                                        op1=mybir.AluOpType.mult)
                ln_tiles[it] = ln_h

            def emit_lnT(it):
                ln_h = ln_tiles[it]
                for ci, (c0, _r0, _rows) in enumerate(CCHUNKS):
                    pt = ps_m.tile([128, 128], F16, name="pt", tag="m")
                    nc.tensor.transpose(pt, ln_h[:, c0:c0 + 128], ident)
                    nc.vector.tensor_copy(lnT[ci][:, it * 128:(it + 1) * 128],
                                          pt)

            def emit_v(it):
                pv = ps_m.tile([128, C], F32, name="pv", tag="m")
                for ci, (_c0, r0, rows) in enumerate(CCHUNKS):
                    nc.tensor.matmul(pv, lnT[ci][r0:128, it * 128:(it + 1) * 128],
                                     wv_sb[ci][r0:128, :],
                                     start=(ci == 0), stop=(ci == 2))
                nc.vector.tensor_add(
                    v_aug[:, it, :, 64:],
                    pv.rearrange("p (h d) -> p h d", h=NH),
                    bv_bc.rearrange("p (h d) -> p h d", h=NH))

            def emit_kT(i, b, tag, w=QB):
                """kT[i] columns [b*w, (b+1)*w)."""
                o0, osz = OCHUNKS[i]
                pk = ps_s.tile([osz, w], F32, name="pk", tag=tag) if tag == "s" \
                    else ps_m.tile([osz, w], F32, name="pk", tag=tag)
                for ci, (_c0, r0, rows) in enumerate(CCHUNKS):
                    nc.tensor.matmul(
                        pk, wk_sb[ci][r0:128, o0:o0 + osz],
                        lnT[ci][r0:128, b * w:(b + 1) * w],
                        start=(ci == 0), stop=(ci == 2))
                nc.vector.tensor_scalar_add(
                    kT[i][:, b * w:(b + 1) * w], pk, bk_col[i])

            def emit_qproj(i, nb, tag):
                o0, osz = OCHUNKS[i]
                pq = ps_s.tile([osz, QB], F32, name="pq", tag=tag) if tag == "s" \
                    else ps_m.tile([osz, QB], F32, name="pq", tag=tag)
                sp, tc0 = (nb * QB) // 1024, (nb * QB) % 1024
                for ci, (_c0, r0, rows) in enumerate(CCHUNKS):
                    nc.tensor.matmul(
                        pq, wq_sb[ci][r0:128, o0:o0 + osz],
                        xTp[ci][sp][r0:128, tc0:tc0 + QB],
                        start=(ci == 0), stop=(ci == 2))
                nc.vector.tensor_copy(qT[i][:, nb * QB:(nb + 1) * QB], pq)

            # ---------------- attention building blocks ----------------
            attnT = {}

            def emit_scores(qb, h, k):
                ht, hr = h // 2, (h % 2) * 64
                ps = ps_s.tile([128, 2 * QB], F32, name="ps", tag="s")
                for qh in range(2):
                    nc.tensor.matmul(
                        ps[:, qh * QB:(qh + 1) * QB],
                        kT[ht][hr:hr + HD, k * 128:(k + 1) * 128],
                        qT[ht][hr:hr + HD,
                               qb * 1024 + qh * QB:qb * 1024 + (qh + 1) * QB],
                        start=True, stop=True)
                se = sexp_p.tile([128, 2 * QB], F16, name="se", tag="sexp")
                nc.scalar.activation(se, ps, mybir.ActivationFunctionType.Exp,
                                     bias=ebias_t, scale=SCALE)
                if dbg and qb == 0 and h == 0 and k == 0:
                    nc.sync.dma_start(out=dbg_d["dbg_se"][:, :], in_=se)
                return se

            def emit_av(pavs, h, k, se):
                for qh in range(2):
                    nc.tensor.matmul(
                        pavs[qh], v_aug[:, k, h, :],
                        se[:, qh * QB:(qh + 1) * QB],
                        start=(k == 0), stop=(k == 7))

            def emit_norm(qb, h, pavs):
                """Release pav fast (reciprocal + value copy), then lazily
                broadcast+multiply into attnT."""
                dst = attnT[qb][h // 2]
                dr = (h % 2) * 64
                for qh in range(2):
                    rec = small_p.tile([1, QB], F32, name="rec", tag="rc")
                    nc.vector.reciprocal_approx_fast(rec, pavs[qh][0:1, :])
                    vcp = vcop_p.tile([64, QB], F16, name="vcp", tag="vc")
                    nc.vector.tensor_copy(vcp, pavs[qh][64:128, :])
                    rb = small_p.tile([HD, QB], F32, name="rb", tag="rb")
                    nc.gpsimd.partition_broadcast(rb, rec)
                    nc.vector.tensor_mul(
                        dst[dr:dr + HD, qh * QB:(qh + 1) * QB], vcp, rb)

            def emit_proj_qs(qb, qs):
                po = ps_m.tile([128, C], F32, name="po", tag="m")
                for ci, (o0, osz) in enumerate(OCHUNKS):
                    nc.tensor.matmul(
                        po, attnT[qb][ci][:, qs * 128:(qs + 1) * 128],
                        wp_o[ci], start=(ci == 0), stop=(ci == 2))
                o_sb = out_p.tile([128, C], F32, name="o_sb", tag="o")
                nc.vector.tensor_add(o_sb, po, bp_bc)
                nc.sync.dma_start(
                    out=out_d[(qb * 8 + qs) * 128:(qb * 8 + qs + 1) * 128, :],
                    in_=o_sb)

            # ---------------- prep queue ----------------
            prep = deque()

            def pump(n):
                for _ in range(n):
                    if prep:
                        prep.popleft()()

            def prep_block(it):
                # conv block it as 5 queue items: 4 shift groups + (lnT+v)
                for s in range(4):
                    prep.append(lambda it=it, s=s: conv_group(it, s))
                prep.append(lambda it=it: (emit_lnT(it), emit_v(it)))

            # ---------------- ramp ----------------
            for it in range(4):
                for s in range(4):
                    conv_group(it, s)
            for it in range(4):
                emit_lnT(it)
                emit_v(it)
            emit_kT(0, 0, "s")
            emit_qproj(0, 0, "s")
            emit_qproj(0, 1, "s")

            # remaining prep, in dependency-safe pump order
            for it in range(4, 8):
                prep_block(it)
            prep.append(lambda: (emit_kT(1, 0, "m"), emit_kT(1, 1, "m")))
            prep.append(lambda: (emit_qproj(1, 0, "m"), emit_qproj(1, 1, "m")))
            prep.append(lambda: (emit_kT(2, 0, "m"), emit_kT(2, 1, "m")))
            prep.append(lambda: (emit_qproj(2, 0, "m"), emit_qproj(2, 1, "m")))
            for nb in range(2, 8):
                for i in range(3):
                    prep.append(lambda i=i, nb=nb: emit_qproj(i, nb, "m"))

            # ---------------- attention ----------------
            for qb in range(4):
                attnT[qb] = [
                    attn_p.tile([osz, 1024], F16, name=f"aT{qb}_{i}",
                                tag=f"attn{i}")
                    for i, (_o0, osz) in enumerate(OCHUNKS)]
                for h in range(NH):
                    pavs = [ps_a.tile([128, QB], F32, name="pav", tag="a")
                            for _ in range(2)]
                    ses = {}
                    if qb == 0 and h == 0:
                        # special pacing: kv chunks 4..7 need conv(4..7),
                        # lnT/v(k) and the kT[0] 128-col piece first.
                        for k in range(4):
                            ses[k] = emit_scores(qb, h, k)
                            if k >= 1:
                                emit_av(pavs, h, k - 1, ses[k - 1])
                        for k in range(4, 8):
                            pump(5)        # conv block k + lnT/v(k)
                            emit_kT(0, k, "m", w=128)
                            ses[k] = emit_scores(qb, h, k)
                            emit_av(pavs, h, k - 1, ses[k - 1])
                        emit_av(pavs, h, 7, ses[7])
                    else:
                        for k in range(8):
                            ses[k] = emit_scores(qb, h, k)
                            if k >= 1:
                                emit_av(pavs, h, k - 1, ses[k - 1])
                            if k % 2 == 1:
                                pump(1)
                        emit_av(pavs, h, 7, ses[7])
                    emit_norm(qb, h, pavs)
                    pump(1)
                if dbg and qb == 0:
                    nc.sync.dma_start(out=dbg_d["dbg_at"][:, :], in_=attnT[0][0])
                    for sp in range(4):
                        nc.sync.dma_start(
                            out=dbg_d["dbg_xt0"][:, sp * 1024:(sp + 1) * 1024],
                            in_=xTp[0][sp])
                        nc.sync.dma_start(
                            out=dbg_d["dbg_xtd0"][:, sp * 1024:(sp + 1) * 1024],
                            in_=xTdp[0][sp])
                    nc.sync.dma_start(out=dbg_d["dbg_ln0"][:, :], in_=lnT[0])
                    nc.sync.dma_start(out=dbg_d["dbg_kt0"][0:128, :], in_=kT[0])
                    nc.sync.dma_start(
                        out=dbg_d["dbg_v"][:, :],
                        in_=v_aug.rearrange("p a b c -> p (a b c)"))
                if qb < 3:
                    for qs in range(8):
                        prep.append(lambda qb=qb, qs=qs: emit_proj_qs(qb, qs))
            pump(len(prep))
            for qs in range(8):
                emit_proj_qs(3, qs)
            if dbg:
                nc.sync.dma_start(out=dbg_d["dbg_qt0"][:, :], in_=qT[0])

    nc.compile()
    return nc


_CACHE = {}


def _get_nc():
    if "nc" not in _CACHE:
        _CACHE["nc"] = build_bass()
    return _CACHE["nc"]


def make_in_maps(x, Wq, Wkv, sr_w, sr_b, ln_g, ln_b, Wp, bp):
    B = x.shape[0]
    f16 = np.float16
    f32 = np.float32
    ln_g = np.asarray(ln_g, f32)
    ln_b = np.asarray(ln_b, f32)
    wk_f = np.asarray(Wkv[:, :C], f32)
    wv_f = np.asarray(Wkv[:, C:], f32)
    wq = np.ascontiguousarray(Wq, dtype=f16)
    # fold LN gamma/beta into the K/V projections:
    #   (ln*g + b) @ W = ln @ (g[:,None]*W) + b @ W
    wk = np.ascontiguousarray(ln_g[:, None] * wk_f, dtype=f16)
    wv = np.ascontiguousarray(ln_g[:, None] * wv_f, dtype=f16)
    bk = np.ascontiguousarray(ln_b @ wk_f, dtype=f32)
    bv = np.ascontiguousarray(ln_b @ wv_f, dtype=f32)
    srw = np.ascontiguousarray(np.asarray(sr_w, dtype=f16).reshape(4 * C, C))
    wp = np.ascontiguousarray(Wp, dtype=f16)
    srb = np.ascontiguousarray(sr_b, dtype=f32)
    bpv = np.ascontiguousarray(bp, dtype=f32)
    # Host-side layout prep: x^T, plus the shift-deinterleaved xd^T for the
    # conv's stationary operand (row order [it(h'//4), dh, dw, h'%4, w'] <-
    # x row (2h'+dh)*64 + 2w'+dw). Pre-transposed so device DMAs are plain
    # contiguous transfers.
    xf = np.asarray(x, dtype=f16)
    xt = np.ascontiguousarray(xf.transpose(0, 2, 1))             # [B, C, N]
    xdt = np.ascontiguousarray(
        xf.reshape(B, 8, 4, 2, 32, 2, C)         # [B, it, h'lo, dh, w', dw, C]
          .transpose(0, 6, 1, 3, 5, 2, 4)         # [B, C, it, dh, dw, h'lo, w']
          .reshape(B, C, N))
    return [
        {"xt": xt[i], "xdt": xdt[i],
         "wq": wq, "wk": wk,
         "wv": wv, "srw": srw, "wp": wp, "srb": srb, "bk": bk,
         "bv": bv, "bp": bpv}
        for i in range(B)
    ]


def kernel(x, Wq, Wkv, sr_w, sr_b, ln_g, ln_b, Wp, bp, H=64, W=64):
    x = np.asarray(x, dtype=np.float32)
    B = x.shape[0]
    assert x.shape == (B, N, C), x.shape
    nc = _get_nc()
    in_maps = make_in_maps(x, Wq, Wkv, sr_w, sr_b, ln_g, ln_b, Wp, bp)
    res = run_bass_kernel_spmd(nc, in_maps, core_ids=list(range(8)))
    out = np.stack([res.results[i]["out"] for i in range(B)], axis=0)
    return out.astype(np.float32)


# revision 29
# speedup vs baseline: 1.2587x; 1.0423x over previous
"""PVT-style spatial-reduction attention on 8 Trainium2 NeuronCores.

Sharding: data-parallel over batch (B=8 -> one batch element per core).
Each core runs the full attention for its batch element; weights are
replicated. No collectives needed.

v2 schedule (ACT-exp is the bottleneck engine; everything hides behind it):
  - x^T arrives as 12 pipelined DMA-transpose pieces on both HWDGE rings.
  - conv2x2s2 contracts strided views of x^T directly as matmul lhsT
    (no patch materialization).
  - attention starts after ~1/4 of the prep (conv blocks 0-3, kT chunk 0,
    qT rows 0:128); remaining conv/lnT/v/kT/qproj/proj work is drip-fed
    into PE slack between score/av matmuls via a prep queue.
  - scores psum ping-pongs in 2x[128,1024] banks; av accumulates in
    2x[128,512]; per-head softmax normalization is taken off the critical
    path by releasing the av psum with a fast reciprocal + value copy,
    then broadcasting/multiplying lazily.
"""

import os
import sys
from collections import deque

import numpy as np

for _p in ("/opt/trn_rl_repo", "/root/.axon_site/_ro/trn_rl_repo"):
    if os.path.isdir(_p) and _p not in sys.path:
        sys.path.append(_p)

import concourse.bacc as bacc
import concourse.bass as bass
import concourse.mybir as mybir
import concourse.tile as tile
from concourse.bass_utils import run_bass_kernel_spmd
from concourse.masks import make_identity

F16 = mybir.dt.float16
F32 = mybir.dt.float32

N = 4096          # q tokens (H*W = 64*64)
C = 320           # model dim
NH = 5            # heads
HD = 64           # head dim
NP = 1024         # kv tokens ((H/2)*(W/2))
QB = 512
LN_EPS = 1e-3
SCALE = HD ** -0.5
EXP_BIAS = -3.0   # constant shift inside exp; cancels in softmax

# contraction chunks over C=320: three 128-row tiles; the last one holds
# c 192:320 and uses rows 64:128 (its top 64 rows overlap chunk 1).
CCHUNKS = [(0, 0, 128), (128, 0, 128), (192, 64, 128)]  # (c_start, row0, rows)
# output chunks over C=320
OCHUNKS = [(0, 128), (128, 128), (256, 64)]


def build_bass(dbg=False):
    nc = bacc.Bacc("TRN2", target_bir_lowering=False, debug=False, num_devices=8)

    xt_d = nc.declare_dram_parameter("xt", [C, N], F16, isOutput=False)
    xdt_d = nc.declare_dram_parameter("xdt", [C, N], F16, isOutput=False)
    wq_d = nc.declare_dram_parameter("wq", [C, C], F16, isOutput=False)
    wk_d = nc.declare_dram_parameter("wk", [C, C], F16, isOutput=False)
    wv_d = nc.declare_dram_parameter("wv", [C, C], F16, isOutput=False)
    srw_d = nc.declare_dram_parameter("srw", [4 * C, C], F16, isOutput=False)
    wp_d = nc.declare_dram_parameter("wp", [C, C], F16, isOutput=False)
    srb_d = nc.declare_dram_parameter("srb", [C], F32, isOutput=False)
    bk_d = nc.declare_dram_parameter("bk", [C], F32, isOutput=False)
    bv_d = nc.declare_dram_parameter("bv", [C], F32, isOutput=False)
    bp_d = nc.declare_dram_parameter("bp", [C], F32, isOutput=False)
    out_d = nc.declare_dram_parameter("out", [N, C], F32, isOutput=True)
    dbg_d = {}
    if dbg:
        for nm, shp in [("dbg_xt0", [128, N]), ("dbg_xtd0", [128, N]),
                        ("dbg_ln0", [128, NP]), ("dbg_kt0", [128, NP]),
                        ("dbg_qt0", [128, N]), ("dbg_v", [128, 8 * NH * 128]),
                        ("dbg_se", [128, 1024]), ("dbg_at", [128, 1024])]:
            dbg_d[nm] = nc.declare_dram_parameter(nm, shp, F16, isOutput=True)

    with tile.TileContext(nc) as tc:
        with (
            tc.tile_pool(name="consts", bufs=1) as consts,
            tc.tile_pool(name="wpool", bufs=1) as wpool,
            tc.tile_pool(name="big", bufs=1) as bigp,
            tc.tile_pool(name="sexp", bufs=18) as sexp_p,
            tc.tile_pool(name="attn", bufs=2) as attn_p,
            tc.tile_pool(name="small", bufs=4) as small_p,
            tc.tile_pool(name="vcop", bufs=4) as vcop_p,
            tc.tile_pool(name="outp", bufs=4) as out_p,
            tc.tile_pool(name="ps_s", bufs=2, space="PSUM") as ps_s,
            tc.tile_pool(name="ps_a", bufs=2, space="PSUM") as ps_a,
            tc.tile_pool(name="ps_m", bufs=2, space="PSUM") as ps_m,
        ):
            # ---------------- DMA: x^T pieces + weights ----------------
            # x^T tiles (c on partitions), filled by 12 transpose pieces
            # (3 c-chunks x 4 token-quarters) so conv/qproj can start after
            # the first quarter instead of after the whole 14us transpose.
            # xTd is the shift-deinterleaved copy (host-permuted xd input,
            # rows = [it, dh, dw, h'%4, w']): the conv's stationary operand
            # becomes a plain contiguous 2D slice (PE weights allow only one
            # free dim). xTd pieces go on the sync ring, xT on the ACT ring.
            # x^T and the shift-deinterleaved xd^T are pre-transposed on the
            # host, so all input DMAs are plain row-contiguous transfers
            # (on-device xbar transposes ran at ~20 GB/s in 640B packets and
            # starved the ramp). Per-piece tiles keep subtile deps exact.
            # xTp[ci][sp] holds tokens [sp*1024, (sp+1)*1024) of x^T chunk ci.
            xTp = [[bigp.tile([128, 1024], F16, name=f"xT{i}_{sp}")
                    for sp in range(4)] for i in range(3)]
            xTdp = [[bigp.tile([128, 1024], F16, name=f"xTd{i}_{sp}")
                     for sp in range(4)] for i in range(3)]

            def xtd_piece(sp):
                for ci, (c0, _r0, _rows) in enumerate(CCHUNKS):
                    nc.sync.dma_start(
                        out=xTdp[ci][sp],
                        in_=xdt_d[c0:c0 + 128, sp * 1024:(sp + 1) * 1024])

            def xt_piece(sp):
                for ci, (c0, _r0, _rows) in enumerate(CCHUNKS):
                    nc.sync.dma_start(
                        out=xTp[ci][sp],
                        in_=xt_d[c0:c0 + 128, sp * 1024:(sp + 1) * 1024])

            # ALL DMAs issue on the sync ring: a dma_start on the ACT engine
            # blocks the ACT FIFO on ring backpressure, starving the exps.
            xtd_piece(0)

            # sr_w in 12 per-(shift, cchunk) tiles; chunk 2 parks its 64
            # rows at partitions 64:128 so matmul bases match the xTd slice.
            srw_sb = []
            for s in range(4):
                per_s = []
                for ci, (c0, r0, rows) in enumerate(CCHUNKS):
                    t = wpool.tile([128, C], F16, name=f"srw{s}_{ci}")
                    nc.sync.dma_start(
                        out=t[r0:128, :],
                        in_=srw_d[s * C + c0 + r0:s * C + c0 + 128, :])
                    per_s.append(t)
                srw_sb.append(per_s)

            def load_w_chunks(dram, name):
                ts = []
                for i, (c0, _r0, rows) in enumerate(CCHUNKS):
                    t = wpool.tile([rows, C], F16, name=f"{name}{i}")
                    nc.sync.dma_start(out=t, in_=dram[c0:c0 + rows, :])
                    ts.append(t)
                return ts

            def bcast(dram_vec, name):
                t = consts.tile([128, C], F32, name=name)
                src = bass.AP(tensor=dram_vec.ap().tensor, offset=0,
                              ap=[[0, 128], [1, C]])
                nc.sync.dma_start(out=t, in_=src)
                return t

            xt_piece(0)
            wq_sb = load_w_chunks(wq_d, "wq")
            srb_bc = bcast(srb_d, "srb_bc")
            bv_bc = bcast(bv_d, "bv_bc")
            bk_col = []
            for i, (o0, osz) in enumerate(OCHUNKS):
                b = wpool.tile([osz, 1], F32, name=f"bk{i}")
                nc.sync.dma_start(out=b, in_=bk_d[o0:o0 + osz].unsqueeze(1))
                bk_col.append(b)
            xtd_piece(1)
            xt_piece(1)
            wk_sb = load_w_chunks(wk_d, "wk")
            wv_sb = load_w_chunks(wv_d, "wv")
            xtd_piece(2)
            xtd_piece(3)
            xt_piece(2)
            xt_piece(3)
            wp_o = []
            for i, (o0, osz) in enumerate(OCHUNKS):
                t = wpool.tile([osz, C], F16, name=f"wp{i}")
                nc.sync.dma_start(out=t, in_=wp_d[o0:o0 + osz, :])
                wp_o.append(t)
            bp_bc = bcast(bp_d, "bp_bc")

            ident = consts.tile([128, 128], F16, name="ident")
            make_identity(nc, ident)
            eps_t = consts.tile([128, 1], F32, name="eps_t")
            nc.vector.memset(eps_t, LN_EPS)
            ebias_t = consts.tile([128, 1], F32, name="ebias_t")
            nc.vector.memset(ebias_t, EXP_BIAS)

            # warm the ACT exp table set during the ramp so the ~2.7us
            # table load doesn't land inside the attention phase.
            warm = small_p.tile([128, 1], F16, name="warm", tag="st")
            nc.scalar.activation(warm, eps_t,
                                 mybir.ActivationFunctionType.Exp)

            # v augmented: [128, kv_chunk(8), head(5), 128] with ones col 0
            # (softmax denominators land on psum partition 0), zeros 1:64,
            # v at 64:128.
            v_aug = bigp.tile([128, 8, NH, 128], F16, name="v_aug")
            nc.vector.memset(v_aug[:, :, :, 0:64], 0.0)
            nc.vector.memset(v_aug[:, :, :, 0:1], 1.0)

            lnT = [bigp.tile([128, NP], F16, name=f"lnT{i}") for i in range(3)]
            kT = [bigp.tile([osz, NP], F16, name=f"kT{i}")
                  for i, (_o0, osz) in enumerate(OCHUNKS)]
            qT = [bigp.tile([osz, N], F16, name=f"qT{i}")
                  for i, (_o0, osz) in enumerate(OCHUNKS)]

            ln_tiles = [None] * 8

            # ---------------- prep building blocks ----------------
            def conv_group(it, s):
                """One shift (dh,dw) of conv block it: 3 accumulating mms.
                xTd columns are [it(8), shift(4), tok'(128)] so the
                stationary operand is a contiguous 2D slice."""
                if s == 0:
                    conv_group.pc = ps_m.tile([128, C], F32, name="pc", tag="m")
                pc = conv_group.pc
                t0 = it * 512 + s * 128
                sp, tc0 = t0 // 1024, t0 % 1024
                for ci, (_c0, r0, rows) in enumerate(CCHUNKS):
                    nc.tensor.matmul(pc, xTdp[ci][sp][r0:128, tc0:tc0 + 128],
                                     srw_sb[s][ci][r0:128, :],
                                     start=(s == 0 and ci == 0),
                                     stop=(s == 3 and ci == 2))
                if s == 3:
                    ln_stats(it, pc)

            I32 = mybir.dt.int32

            def ln_stats(it, pc):
                nc.vector.tensor_add(pc, pc, srb_bc)
                stats = small_p.tile([128, 6], F32, name="stats", tag="st")
                nc.vector.bn_stats(stats, pc)
                mv = small_p.tile([128, 2], F32, name="mv", tag="st")
                nc.vector.bn_aggr(mv, stats)
                # rstd = rsqrt(var+eps) fully on DVE (Schraudolph seed + 2
                # Newton steps) so ACT runs nothing but Exp (its table set
                # has no sqrt; mixing forces ~2.6us table swaps per LN).
                s = small_p.tile([128, 8], F32, name="nrs", tag="st")
                nc.vector.tensor_scalar_add(s[:, 0:1], mv[:, 1:2], eps_t)
                nc.vector.tensor_scalar(
                    s[:, 1:2].bitcast(I32), s[:, 0:1].bitcast(I32),
                    1, -1,
                    op0=mybir.AluOpType.logical_shift_right,
                    op1=mybir.AluOpType.bitwise_xor)
                nc.vector.tensor_scalar_add(
                    s[:, 2:3].bitcast(I32), s[:, 1:2].bitcast(I32),
                    0x5F3759DF + 1)
                y = s[:, 2:3]
                for c in (4, 7):
                    nc.vector.tensor_mul(s[:, 3:4], y, y)
                    nc.vector.tensor_mul(s[:, 5:6], s[:, 3:4], s[:, 0:1])
                    nc.vector.tensor_scalar(
                        s[:, 6:7], s[:, 5:6], -0.5, 1.5,
                        op0=mybir.AluOpType.mult, op1=mybir.AluOpType.add)
                    nc.vector.tensor_mul(s[:, c:c + 1], s[:, 6:7], y)
                    y = s[:, c:c + 1]
                ln_h = small_p.tile([128, C], F16, name="ln_h", tag="lnf")
                nc.vector.tensor_scalar(ln_h, pc, mv[:, 0:1], y,
                                        op0=mybir.AluOpType.subtract,
                                        op1=mybir.AluOpType.mult)
                ln_tiles[it] = ln_h

            def emit_lnT(it):
                ln_h = ln_tiles[it]
                for ci, (c0, _r0, _rows) in enumerate(CCHUNKS):
                    pt = ps_m.tile([128, 128], F16, name="pt", tag="m")
                    nc.tensor.transpose(pt, ln_h[:, c0:c0 + 128], ident)
                    nc.vector.tensor_copy(lnT[ci][:, it * 128:(it + 1) * 128],
                                          pt)

            def emit_v(it):
                pv = ps_m.tile([128, C], F32, name="pv", tag="m")
                for ci, (_c0, r0, rows) in enumerate(CCHUNKS):
                    nc.tensor.matmul(pv, lnT[ci][r0:128, it * 128:(it + 1) * 128],
                                     wv_sb[ci][r0:128, :],
                                     start=(ci == 0), stop=(ci == 2))
                nc.vector.tensor_add(
                    v_aug[:, it, :, 64:],
                    pv.rearrange("p (h d) -> p h d", h=NH),
                    bv_bc.rearrange("p (h d) -> p h d", h=NH))

            def emit_kT(i, b, tag, w=QB):
                """kT[i] columns [b*w, (b+1)*w)."""
                o0, osz = OCHUNKS[i]
                pk = ps_s.tile([osz, w], F32, name="pk", tag=tag) if tag == "s" \
                    else ps_m.tile([osz, w], F32, name="pk", tag=tag)
                for ci, (_c0, r0, rows) in enumerate(CCHUNKS):
                    nc.tensor.matmul(
                        pk, wk_sb[ci][r0:128, o0:o0 + osz],
                        lnT[ci][r0:128, b * w:(b + 1) * w],
                        start=(ci == 0), stop=(ci == 2))
                nc.vector.tensor_scalar_add(
                    kT[i][:, b * w:(b + 1) * w], pk, bk_col[i])

            def emit_qproj(i, nb, tag):
                o0, osz = OCHUNKS[i]
                pq = ps_s.tile([osz, QB], F32, name="pq", tag=tag) if tag == "s" \
                    else ps_m.tile([osz, QB], F32, name="pq", tag=tag)
                sp, tc0 = (nb * QB) // 1024, (nb * QB) % 1024
                for ci, (_c0, r0, rows) in enumerate(CCHUNKS):
                    nc.tensor.matmul(
                        pq, wq_sb[ci][r0:128, o0:o0 + osz],
                        xTp[ci][sp][r0:128, tc0:tc0 + QB],
                        start=(ci == 0), stop=(ci == 2))
                nc.vector.tensor_copy(qT[i][:, nb * QB:(nb + 1) * QB], pq)

            # ---------------- attention building blocks ----------------
            attnT = {}

            def emit_scores(qb, h, k):
                ht, hr = h // 2, (h % 2) * 64
                ps = ps_s.tile([128, 2 * QB], F32, name="ps", tag="s")
                for qh in range(2):
                    nc.tensor.matmul(
                        ps[:, qh * QB:(qh + 1) * QB],
                        kT[ht][hr:hr + HD, k * 128:(k + 1) * 128],
                        qT[ht][hr:hr + HD,
                               qb * 1024 + qh * QB:qb * 1024 + (qh + 1) * QB],
                        start=True, stop=True)
                se = sexp_p.tile([128, 2 * QB], F16, name="se", tag="sexp")
                nc.scalar.activation(se, ps, mybir.ActivationFunctionType.Exp,
                                     bias=ebias_t, scale=SCALE)
                if dbg and qb == 0 and h == 0 and k == 0:
                    nc.sync.dma_start(out=dbg_d["dbg_se"][:, :], in_=se)
                return se

            def emit_av(pavs, h, k, se):
                for qh in range(2):
                    nc.tensor.matmul(
                        pavs[qh], v_aug[:, k, h, :],
                        se[:, qh * QB:(qh + 1) * QB],
                        start=(k == 0), stop=(k == 7))

            def emit_norm(qb, h, pavs):
                """Release pav fast (reciprocal + value copy), then lazily
                broadcast+multiply into attnT."""
                dst = attnT[qb][h // 2]
                dr = (h % 2) * 64
                for qh in range(2):
                    rec = small_p.tile([1, QB], F32, name="rec", tag="rc")
                    nc.vector.reciprocal_approx_fast(rec, pavs[qh][0:1, :])
                    vcp = vcop_p.tile([64, QB], F16, name="vcp", tag="vc")
                    nc.vector.tensor_copy(vcp, pavs[qh][64:128, :])
                    rb = small_p.tile([HD, QB], F32, name="rb", tag="rb")
                    nc.gpsimd.partition_broadcast(rb, rec)
                    nc.vector.tensor_mul(
                        dst[dr:dr + HD, qh * QB:(qh + 1) * QB], vcp, rb)

            def emit_proj_qs(qb, qs):
                po = ps_m.tile([128, C], F32, name="po", tag="m")
                for ci, (o0, osz) in enumerate(OCHUNKS):
                    nc.tensor.matmul(
                        po, attnT[qb][ci][:, qs * 128:(qs + 1) * 128],
                        wp_o[ci], start=(ci == 0), stop=(ci == 2))
                o_sb = out_p.tile([128, C], F32, name="o_sb", tag="o")
                nc.vector.tensor_add(o_sb, po, bp_bc)
                nc.sync.dma_start(
                    out=out_d[(qb * 8 + qs) * 128:(qb * 8 + qs + 1) * 128, :],
                    in_=o_sb)

            # ---------------- prep queue ----------------
            prep = deque()

            def pump(n):
                for _ in range(n):
                    if prep:
                        prep.popleft()()

            def prep_block(it):
                # conv block it as 5 queue items: 4 shift groups + (lnT+v)
                for s in range(4):
                    prep.append(lambda it=it, s=s: conv_group(it, s))
                prep.append(lambda it=it: (emit_lnT(it), emit_v(it)))

            # ---------------- ramp ----------------
            for it in range(4):
                for s in range(4):
                    conv_group(it, s)
            for it in range(4):
                emit_lnT(it)
                emit_v(it)
            emit_kT(0, 0, "s")
            emit_qproj(0, 0, "s")
            emit_qproj(0, 1, "s")

            # remaining prep, in dependency-safe pump order
            for it in range(4, 8):
                prep_block(it)
            prep.append(lambda: emit_kT(1, 0, "m"))
            prep.append(lambda: emit_kT(1, 1, "m"))
            prep.append(lambda: emit_qproj(1, 0, "m"))
            prep.append(lambda: emit_qproj(1, 1, "m"))
            prep.append(lambda: emit_kT(2, 0, "m"))
            prep.append(lambda: emit_kT(2, 1, "m"))
            prep.append(lambda: emit_qproj(2, 0, "m"))
            prep.append(lambda: emit_qproj(2, 1, "m"))
            for nb in range(2, 8):
                for i in range(3):
                    prep.append(lambda i=i, nb=nb: emit_qproj(i, nb, "m"))

            # ---------------- attention ----------------
            for qb in range(4):
                attnT[qb] = [
                    attn_p.tile([osz, 1024], F16, name=f"aT{qb}_{i}",
                                tag=f"attn{i}")
                    for i, (_o0, osz) in enumerate(OCHUNKS)]
                for h in range(NH):
                    pavs = [ps_a.tile([128, QB], F32, name="pav", tag="a")
                            for _ in range(2)]
                    ses = {}
                    if qb == 0 and h == 0:
                        # special pacing: kv chunks 4..7 need conv(4..7),
                        # lnT/v(k) and the kT[0] 128-col piece first.
                        for k in range(4):
                            ses[k] = emit_scores(qb, h, k)
                            if k >= 1:
                                emit_av(pavs, h, k - 1, ses[k - 1])
                        for k in range(4, 8):
                            pump(5)        # conv block k + lnT/v(k)
                            emit_kT(0, k, "m", w=128)
                            ses[k] = emit_scores(qb, h, k)
                            emit_av(pavs, h, k - 1, ses[k - 1])
                        emit_av(pavs, h, 7, ses[7])
                    else:
                        for k in range(8):
                            ses[k] = emit_scores(qb, h, k)
                            if k >= 1:
                                emit_av(pavs, h, k - 1, ses[k - 1])
                            if k % 2 == 1:
                                pump(1)
                        emit_av(pavs, h, 7, ses[7])
                    emit_norm(qb, h, pavs)
                    pump(1)
                if dbg and qb == 0:
                    nc.sync.dma_start(out=dbg_d["dbg_at"][:, :], in_=attnT[0][0])
                    for sp in range(4):
                        nc.sync.dma_start(
                            out=dbg_d["dbg_xt0"][:, sp * 1024:(sp + 1) * 1024],
                            in_=xTp[0][sp])
                        nc.sync.dma_start(
                            out=dbg_d["dbg_xtd0"][:, sp * 1024:(sp + 1) * 1024],
                            in_=xTdp[0][sp])
                    nc.sync.dma_start(out=dbg_d["dbg_ln0"][:, :], in_=lnT[0])
                    nc.sync.dma_start(out=dbg_d["dbg_kt0"][0:128, :], in_=kT[0])
                    nc.sync.dma_start(
                        out=dbg_d["dbg_v"][:, :],
                        in_=v_aug.rearrange("p a b c -> p (a b c)"))
                if qb < 3:
                    for qs in range(8):
                        prep.append(lambda qb=qb, qs=qs: emit_proj_qs(qb, qs))
            pump(len(prep))
            for qs in range(8):
                emit_proj_qs(3, qs)
            if dbg:
                nc.sync.dma_start(out=dbg_d["dbg_qt0"][:, :], in_=qT[0])

    nc.compile()
    return nc


_CACHE = {}


def _get_nc():
    if "nc" not in _CACHE:
        _CACHE["nc"] = build_bass()
    return _CACHE["nc"]


def make_in_maps(x, Wq, Wkv, sr_w, sr_b, ln_g, ln_b, Wp, bp):
    B = x.shape[0]
    f16 = np.float16
    f32 = np.float32
    ln_g = np.asarray(ln_g, f32)
    ln_b = np.asarray(ln_b, f32)
    wk_f = np.asarray(Wkv[:, :C], f32)
    wv_f = np.asarray(Wkv[:, C:], f32)
    wq = np.ascontiguousarray(Wq, dtype=f16)
    # fold LN gamma/beta into the K/V projections:
    #   (ln*g + b) @ W = ln @ (g[:,None]*W) + b @ W
    wk = np.ascontiguousarray(ln_g[:, None] * wk_f, dtype=f16)
    wv = np.ascontiguousarray(ln_g[:, None] * wv_f, dtype=f16)
    bk = np.ascontiguousarray(ln_b @ wk_f, dtype=f32)
    bv = np.ascontiguousarray(ln_b @ wv_f, dtype=f32)
    srw = np.ascontiguousarray(np.asarray(sr_w, dtype=f16).reshape(4 * C, C))
    wp = np.ascontiguousarray(Wp, dtype=f16)
    srb = np.ascontiguousarray(sr_b, dtype=f32)
    bpv = np.ascontiguousarray(bp, dtype=f32)
    # Host-side layout prep: x^T, plus the shift-deinterleaved xd^T for the
    # conv's stationary operand (row order [it(h'//4), dh, dw, h'%4, w'] <-
    # x row (2h'+dh)*64 + 2w'+dw). Pre-transposed so device DMAs are plain
    # contiguous transfers.
    xf = np.asarray(x, dtype=f16)
    xt = np.ascontiguousarray(xf.transpose(0, 2, 1))             # [B, C, N]
    xdt = np.ascontiguousarray(
        xf.reshape(B, 8, 4, 2, 32, 2, C)         # [B, it, h'lo, dh, w', dw, C]
          .transpose(0, 6, 1, 3, 5, 2, 4)         # [B, C, it, dh, dw, h'lo, w']
          .reshape(B, C, N))
    return [
        {"xt": xt[i], "xdt": xdt[i],
         "wq": wq, "wk": wk,
         "wv": wv, "srw": srw, "wp": wp, "srb": srb, "bk": bk,
         "bv": bv, "bp": bpv}
        for i in range(B)
    ]


def kernel(x, Wq, Wkv, sr_w, sr_b, ln_g, ln_b, Wp, bp, H=64, W=64):
    x = np.asarray(x, dtype=np.float32)
    B = x.shape[0]
    assert x.shape == (B, N, C), x.shape
    nc = _get_nc()
    in_maps = make_in_maps(x, Wq, Wkv, sr_w, sr_b, ln_g, ln_b, Wp, bp)
    res = run_bass_kernel_spmd(nc, in_maps, core_ids=list(range(8)))
    out = np.stack([res.results[i]["out"] for i in range(B)], axis=0)
    return out.astype(np.float32)
